# revision 33
# baseline (speedup 1.0000x reference)
"""MultiHeadSelfAttention Trainium2 Bass kernel, 8-core SPMD.

Reference:
  q,k,v = einsum('bnd,hkd->bhnk', x, W_{q,k,v});  s = q k^T / sqrt(dk)
  p = softmax(s); out = (p v).transpose -> [B,N,H*DK]; out @ Wo^T + bo

Sharding: head-pair per core (core c owns heads 2c, 2c+1, all batches).
Each core computes a partial output projection over its 128 d-columns of
Wo; host sums the 8 partials and adds the bias.

Numerics: matmuls run as float32r (fp22 operand reads, fp32 PSUM accum).
Softmax row-max comes from an f32r scores pass ([q,m] orientation)
reduced on DVE via tensor_scalar(op1=min) accum; the -max is folded into
the f32r S^T pass as a 65th contraction row, so exp needs no per-q bias.
Denominators come from a ones column appended to V; normalization is
DVE reciprocal -> PE rank-1 broadcast -> ACT stage -> DVE multiply.

Schedule: three interleaved instruction streams per batch window --
attention(b) [st matmuls + exp + pv one step behind], prep(b+1)
[x loads, projections, v transpose, max sweeps], outproj(b-1). All
[128,512] PSUM tiles (S^T halves, sweep chunks, projection slices,
v-transposes, outproj halves) share ONE 6-buffer ring pool, so
execution advances in lockstep with the interleaved emission order --
without the ring, the readiness-greedy scheduler lets attention finish
early and the window ends in a DVE-only sweep-reduce dead zone.
Partition-shift/row DMAs ride the gpsimd (Pool) queue so they never
block the SP DMA stream.
"""
import sys

sys.path.insert(0, "/opt/trn_rl_repo")

from itertools import islice

import numpy as np

import concourse.bass as bass
import concourse.mybir as mybir
import concourse.tile as tile
from concourse import bacc
from concourse.bass_utils import run_bass_kernel_spmd
from concourse.masks import make_identity

B, N, D = 4, 2048, 1024
H, DK = 16, 64
NCORES = 8
HPC = H // NCORES          # heads per core = 2
DP = HPC * DK              # d-slice per core = 128
SCALE = 1.0 / float(np.sqrt(DK))

F32 = mybir.dt.float32
F32R = mybir.dt.float32r
BF16 = mybir.dt.bfloat16

NQT = N // 128             # 16 q tiles per head
NMC = N // 128             # 16 m chunks per head
NHALF = N // 1024          # 2 halves (1024-wide)


def r(ap):
    return ap.bitcast(F32R)


def build_program():
    nc = bacc.Bacc("TRN2", target_bir_lowering=False, debug=False,
                   enable_asserts=False, num_devices=NCORES)

    xT_d = nc.dram_tensor("xT", [B, D, N], F32, kind="ExternalInput")
    wq_d = nc.dram_tensor("wq", [D, DP], F32, kind="ExternalInput")
    wk_d = nc.dram_tensor("wk", [D, DP], F32, kind="ExternalInput")
    wv_d = nc.dram_tensor("wv", [D, DP], F32, kind="ExternalInput")
    wo_d = nc.dram_tensor("wo", [DP, D], F32, kind="ExternalInput")
    ones_d = nc.dram_tensor("ones", [128, N], F32, kind="ExternalInput")
    part_d = nc.dram_tensor("partial", [B, N, D], F32, kind="ExternalOutput")

    with tile.TileContext(nc) as tc:
        build_tile_kernel(nc, tc, xT_d, wq_d, wk_d, wv_d, wo_d, ones_d, part_d)
    nc.compile()
    return nc


def build_tile_kernel(nc, tc, xT_d, wq_d, wk_d, wv_d, wo_d, ones_d, part_d):
    from contextlib import ExitStack
    ctx = ExitStack()
    with ctx:
        # ---- persistent tiles ----
        wpool = ctx.enter_context(tc.tile_pool(name="w", bufs=1))
        # weights stored chunk-major along free dim: [128, 8*128]
        w_sb = {}
        for name, dram in (("wq", wq_d), ("wk", wk_d), ("wv", wv_d)):
            t = wpool.tile([128, D // 128 * DP], F32R, tag=name)
            nc.sync.dma_start(
                out=t[:].rearrange("p (c m) -> p c m", m=DP),
                in_=dram.ap().rearrange("(c p) m -> p c m", p=128).bitcast(F32R),
            )
            w_sb[name] = t
        wo_sb = wpool.tile([DP, D], F32R, tag="wo")
        nc.sync.dma_start(out=wo_sb[:], in_=wo_d.ap()[:].bitcast(F32R))
        id_sb = wpool.tile([128, 128], F32, tag="ident")
        make_identity(nc, id_sb[:])
        ones_sb = wpool.tile([1, 128], F32R, tag="onesrow")
        nc.sync.dma_start(out=ones_sb[:],
                          in_=ones_d.ap()[0:1, 0:128].bitcast(F32R))

        # ---- pools ----
        # PSUM (8 banks): ring 6x1 bank [128,512] shared by every
        # producer/consumer pair; oa 1x2 banks (PV accumulator).
        xt_pool = ctx.enter_context(tc.tile_pool(name="xt", bufs=8))
        ring = ctx.enter_context(tc.tile_pool(name="ring", bufs=6,
                                              space="PSUM"))
        oap = ctx.enter_context(tc.tile_pool(name="oap", bufs=1,
                                             space="PSUM"))
        augp = ctx.enter_context(tc.tile_pool(name="aug", bufs=7))
        vsbp = ctx.enter_context(tc.tile_pool(name="vsb", bufs=1))
        vaugp = ctx.enter_context(tc.tile_pool(name="vaug", bufs=3))
        pp = ctx.enter_context(tc.tile_pool(name="psb", bufs=4))
        attp = ctx.enter_context(tc.tile_pool(name="att", bufs=2))
        tmpp = ctx.enter_context(tc.tile_pool(name="tmp", bufs=2))
        scrp = ctx.enter_context(tc.tile_pool(name="scr", bufs=3))
        nmp = ctx.enter_context(tc.tile_pool(name="nm", bufs=4))
        qkfp = ctx.enter_context(tc.tile_pool(name="qkf", bufs=2))
        outp = ctx.enter_context(tc.tile_pool(name="out", bufs=2))

        state = {}   # per-batch live tiles

        def rtile(name):
            return ring.tile([128, 512], F32, tag="ring", name=name)

        def sweep_qt(h, qt, q_aug, k_aug, negmax, staged):
            """One q-tile of the max sweep: 4 scores chunks + row-max."""
            parts = nmp.tile([128, 4], F32, tag="nmparts")
            for ms in range(4):
                sp = rtile("sq")
                nc.tensor.matmul(
                    sp[:],
                    q_aug[h][0:DK, qt * 128:(qt + 1) * 128],
                    k_aug[h][0:DK, ms * 512:(ms + 1) * 512],
                    start=True, stop=True,
                )
                if ms not in staged:
                    # DVE: fused negate + min-accum straight from PSUM
                    scr = scrp.tile([128, 512], BF16, tag="scr")
                    nc.vector.tensor_scalar(
                        scr[:], sp[:], -1.0, None,
                        mybir.AluOpType.mult, mybir.AluOpType.min,
                        accum_out=parts[:, ms:ms + 1],
                    )
                else:
                    # ACT stages bf16 to SBUF; the DVE reduce then runs in
                    # 4x bf16-SBUF mode (DVE is the scarcer engine)
                    scr = scrp.tile([128, 512], BF16, tag="scr")
                    nc.scalar.activation(
                        scr[:], sp[:],
                        mybir.ActivationFunctionType.Copy,
                        bias=0.0, scale=-1.0)
                    scr2 = scrp.tile([128, 512], BF16, tag="scr",
                                     name="scr2")
                    nc.vector.tensor_scalar(
                        scr2[:], scr[:], 0.0, None,
                        mybir.AluOpType.add, mybir.AluOpType.min,
                        accum_out=parts[:, ms:ms + 1],
                    )
            pdump = nmp.tile([128, 4], F32, tag="nmparts", name="pdump")
            nc.vector.tensor_scalar(
                pdump[:], parts[:, 0:4], 0.0, None,
                mybir.AluOpType.add, mybir.AluOpType.min,
                accum_out=negmax[:, qt:qt + 1],
            )

        def finish_negmax(h, q_aug, negmax):
            """[128,16] -max columns -> aug row 64 via DVE transposes+DMA."""
            nm_t = nmp.tile([32, 128], F32, tag="nmt", name=f"nmt{h}")
            for i in range(4):
                nc.vector.transpose(nm_t[0:32, 32 * i:32 * i + 32],
                                    negmax[32 * i:32 * i + 32, 0:32])
            nc.gpsimd.dma_start(out=q_aug[h][64:65, :],
                                in_=nm_t[0:NQT, :].bitcast(F32R))

        def prep_gen(b, stage_all):
            """Prep for batch b: x loads, projections, v transpose, max
            sweeps. Yields between small units."""
            q_aug = [augp.tile([65, N], F32R, tag="aug", name=f"qaug{h}")
                     for h in range(HPC)]
            k_aug = [augp.tile([65, N], F32R, tag="aug", name=f"kaug{h}")
                     for h in range(HPC)]
            q_f32 = qkfp.tile([128, N], F32, tag="qkf")
            k_f32 = qkfp.tile([128, N], F32, tag="qkf")
            v_sb = vsbp.tile([128, N], F32, tag="vsb")
            st = state[b] = {"q_aug": q_aug, "k_aug": k_aug}

            def evac_qk(aug0, f32stage):
                def evac(psq, half, ns):
                    sl = slice(half * 1024 + ns * 512,
                               half * 1024 + (ns + 1) * 512)
                    # head 0 straight to its aug tile (ACT); head 1
                    # staged (DVE) then partition-shift DMA'd -- one copy
                    # per engine so the ring tile's readers overlap and
                    # neither engine eats both
                    nc.scalar.copy(aug0[0:64, sl], psq[0:64, :])
                    nc.vector.tensor_copy(f32stage[64:128, sl],
                                          psq[64:128, :])
                return evac

            def evac_v(psq, half, ns):
                sl = slice(half * 1024 + ns * 512,
                           half * 1024 + (ns + 1) * 512)
                nc.scalar.copy(v_sb[:, sl], psq[:])

            evacs = [("wq", evac_qk(q_aug[0], q_f32)),
                     ("wk", evac_qk(k_aug[0], k_f32)),
                     ("wv", evac_v)]
            for half in range(NHALF):
                xts = []
                for ch in range(8):
                    xt = xt_pool.tile([128, 1024], F32R, tag="xt")
                    nc.sync.dma_start(
                        out=xt[:],
                        in_=xT_d.ap()[b, ch * 128:(ch + 1) * 128,
                                      half * 1024:(half + 1) * 1024
                                      ].bitcast(F32R),
                    )
                    xts.append(xt)
                    if ch % 2 == 1:
                        yield 0.2
                for tname, evac in evacs:
                    for ns in range(2):
                        psq = rtile("prj")
                        for ch in range(8):
                            nc.tensor.matmul(
                                psq[:],
                                r(w_sb[tname][:, ch * DP:(ch + 1) * DP]),
                                r(xts[ch][:, ns * 512:(ns + 1) * 512]),
                                start=(ch == 0), stop=(ch == 7),
                            )
                        evac(psq, half, ns)
                        yield 0.6

            # head-1 partition shifts + ones rows (gpsimd queue so the SP
            # DMA stream never blocks on aug-tile rotation)
            nc.gpsimd.dma_start(out=q_aug[1][0:64, :],
                                in_=q_f32[64:128, :].bitcast(F32R))
            nc.gpsimd.dma_start(out=k_aug[1][0:64, :],
                                in_=k_f32[64:128, :].bitcast(F32R))
            for h in range(HPC):
                nc.gpsimd.dma_start(out=k_aug[h][64:65, :],
                                    in_=ones_d.ap()[0:1, :].bitcast(F32R))
            yield 0.2

            # v transpose -> v_aug chunks [v^T(64) | ones(64)]
            v_aug = st["v_aug"] = []

            def vaug_head(h):
                va = vaugp.tile([128, NMC * 128], F32R, tag="vaug",
                                name=f"vaug{h}")
                v_aug.append(va)
                hs = slice(h * DK, (h + 1) * DK)
                nc.sync.dma_start(
                    out=va[:].rearrange("p (c w) -> p c w", w=128)[:, :, DK:],
                    in_=ones_d.ap()[:, 0:NMC * DK].rearrange(
                        "p (c w) -> p c w", w=DK).bitcast(F32R))
                for g in range(4):
                    vt_ps = rtile("vt_ps")
                    for j in range(4):
                        mc = g * 4 + j
                        nc.tensor.transpose(
                            vt_ps[:, j * 128:j * 128 + DK],
                            v_sb[hs, mc * 128:(mc + 1) * 128],
                            id_sb[hs, hs])
                    nc.vector.tensor_copy(
                        va[:].rearrange("p (c w) -> p c w",
                                        w=128)[:, g * 4:(g + 1) * 4, 0:DK],
                        vt_ps[:].rearrange("p (c w) -> p c w",
                                           w=128)[:, :, 0:DK])
                    yield 0.6

            yield from vaug_head(0)

            # max sweeps: head 0, head 1; v_aug head 1 last (its pool slot
            # frees only after attention(b-1) head 0 finishes)
            for h in range(HPC):
                negmax = nmp.tile([128, 32], F32, tag="nm",
                                  name=f"negmax{h}")
                nc.vector.memset(negmax[:, NQT:], 0.0)
                for qt in range(NQT):
                    if stage_all:
                        # prologue head-0 sweeps run solo: balance point is
                        # ~2.4 ACT-staged chunks per q-tile, so alternate
                        # 2/3; head 1 overlaps batch-0 exp, keep ACT light
                        if h == 0:
                            staged = (1, 3) if qt % 2 == 0 else (1, 2, 3)
                        else:
                            staged = (1, 3)
                    else:
                        staged = (3,) if qt % 2 else ()
                    sweep_qt(h, qt, q_aug, k_aug, negmax, staged)
                    yield 2.0
                finish_negmax(h, q_aug, negmax)
                yield 0.5
            yield from vaug_head(1)

        def attn_gen(b):
            """Attention for batch b. PV runs one m-chunk behind exp so the
            PE stream never parks on a fresh exp."""
            q_aug, k_aug = state[b]["q_aug"], state[b]["k_aug"]
            v_aug = state[b]["v_aug"]
            att = attp.tile([128, N], F32R, tag="att")
            state[b]["att"] = att

            def emit_pv(h, oa, ps, mc):
                for qs in range(2):
                    nc.tensor.matmul(
                        oa[:, qs * 512:(qs + 1) * 512],
                        r(v_aug[h][:, mc * 128:(mc + 1) * 128]),
                        r(ps[qs][:]),
                        start=(mc == 0), stop=(mc == NMC - 1),
                    )

            for h in range(HPC):
                for qh in range(NHALF):
                    qsl = slice(qh * 1024, (qh + 1) * 1024)
                    oa = oap.tile([128, 1024], F32, tag="oa", name="oa")
                    prev = None  # (p_pair, mc) pending PV
                    for mc in range(NMC):
                        ps = []
                        for qs in range(2):
                            stt = rtile("st")
                            nc.tensor.matmul(
                                stt[:],
                                r(k_aug[h][:, mc * 128:(mc + 1) * 128]),
                                r(q_aug[h][:, qh * 1024 + qs * 512:
                                           qh * 1024 + (qs + 1) * 512]),
                                start=True, stop=True,
                            )
                            p_sb = pp.tile([128, 512], F32R, tag="psb")
                            nc.scalar.activation(
                                p_sb[:], stt[:],
                                mybir.ActivationFunctionType.Exp,
                                bias=0.0, scale=SCALE)
                            ps.append(p_sb)
                        if prev is not None:
                            emit_pv(h, oa, *prev)
                        prev = (ps, mc)
                        yield
                    emit_pv(h, oa, *prev)
                    # normalize: att rows = oa[0:64] * (1/denom); denom on
                    # oa[64:65] via the ones columns of v_aug. DVE has no
                    # divide ISA op and engines read at most one PSUM
                    # operand, so: DVE reciprocal -> PE rank-1 broadcast
                    # (ring) -> ACT copy to SBUF -> DVE mult.
                    recip = tmpp.tile([1, 1024], F32R, tag="recip")
                    with nc.allow_low_precision(reason="f32r is 4-byte"):
                        nc.vector.reciprocal(recip[:], oa[64:65, :])
                    rbc = tmpp.tile([64, 1024], F32R, tag="atmp",
                                    name="rbc")
                    for qs in range(2):
                        rb = rtile("rb")
                        nc.tensor.matmul(
                            rb[0:64, :],
                            r(ones_sb[0:1, 0:64]),
                            r(recip[0:1, qs * 512:(qs + 1) * 512]),
                            start=True, stop=True,
                        )
                        nc.scalar.copy(rbc[:, qs * 512:(qs + 1) * 512],
                                       rb[0:64, :])
                    if h == 0:
                        nc.vector.tensor_tensor(
                            att[0:64, qsl], oa[0:64, :], rbc[:],
                            mybir.AluOpType.mult)
                    else:
                        atmp = tmpp.tile([64, 1024], F32R, tag="atmp",
                                         name="atmp")
                        nc.vector.tensor_tensor(
                            atmp[:], oa[0:64, :], rbc[:],
                            mybir.AluOpType.mult)
                        # partition shift 0-63 -> 64-127
                        nc.gpsimd.dma_start(out=att[64:128, qsl],
                                            in_=atmp[:])
                    yield

        def outproj_gen(b, dve_only=False):
            att = state[b]["att"]
            for nt in range(N // 128):
                ostg = outp.tile([128, 1024], F32, tag="ostg")
                for es in range(2):
                    op = rtile("op")
                    nc.tensor.matmul(
                        op[:],
                        r(att[:, nt * 128:(nt + 1) * 128]),
                        r(wo_sb[:, es * 512:(es + 1) * 512]),
                        start=True, stop=True,
                    )
                    if es == 0 and not dve_only:
                        # ACT takes one half except in the last window,
                        # where exp already saturates ACT
                        nc.scalar.copy(ostg[:, 0:512], op[:])
                    else:
                        nc.vector.tensor_copy(
                            ostg[:, es * 512:(es + 1) * 512], op[:])
                nc.sync.dma_start(
                    out=part_d.ap()[b, nt * 128:(nt + 1) * 128, :],
                    in_=ostg[:],
                )
                yield

        # ---- drive the interleaved windows ----
        A_STEPS = HPC * NHALF * (NMC + 1)          # 68
        P_WEIGHT = 8 * 0.2 + 12 * 0.6 + 0.2 + 8 * 0.6 + 32 * 2.0 + 2 * 0.5
        # prologue: prep(0) alone through sweep head 0 (step 42), then its
        # remainder interleaves with early attention(0)
        p0 = prep_gen(0, stage_all=True)
        for _ in islice(p0, 42):
            pass
        a_cur = attn_gen(0)
        a_consumed = 0
        for _ in p0:
            next(a_cur, None)
            a_consumed += 1

        _sent = object()
        for b in range(B):
            a = a_cur
            a_rem = max(1, A_STEPS - (a_consumed if b == 0 else 0))
            p = prep_gen(b + 1, stage_all=False) if b + 1 < B else None
            o = outproj_gen(b - 1, dve_only=(b == B - 1)) if b >= 1 else None
            o_self = None
            i = 0

            def pull_a():
                # one attention step plus its share of the outproj streams
                nonlocal i, o_self
                if next(a, _sent) is _sent:
                    return False
                if o is not None and i % 4 == 0:
                    next(o, None)
                if b == B - 1 and i >= a_rem - 16:
                    # last window: batch-3 outproj columns for the first
                    # q-half become valid once head-1/qh0 finishes
                    if o_self is None:
                        o_self = outproj_gen(b, dve_only=True)
                    if i % 2 == 0:
                        next(o_self, None)
                i += 1
                return True

            if p is not None:
                # pace attention by prep weight: a sweep step carries 4x
                # the ring tiles of a projection step
                acc = 0.0
                alive = True
                for w in p:
                    acc += w * (a_rem / P_WEIGHT)
                    while acc >= 1.0:
                        acc -= 1.0
                        if alive:
                            alive = pull_a()
            while pull_a():
                pass
            if o is not None:
                for _ in o:
                    pass
            a_cur = attn_gen(b + 1) if b + 1 < B else None
        for _ in (o_self or outproj_gen(B - 1, dve_only=True)):
            pass


_PROGRAM = None


def _get_program():
    global _PROGRAM
    if _PROGRAM is None:
        _PROGRAM = build_program()
    return _PROGRAM


_ONES = np.ones((128, N), np.float32)


def make_in_maps(x, W_q, W_k, W_v, Wo_w):
    xT = np.ascontiguousarray(np.transpose(
        np.asarray(x, np.float32), (0, 2, 1)))
    in_maps = []
    for c in range(NCORES):
        hs = slice(HPC * c, HPC * (c + 1))
        wq = np.ascontiguousarray(
            np.asarray(W_q[hs], np.float32).reshape(DP, D).T)
        wk = np.ascontiguousarray(
            np.asarray(W_k[hs], np.float32).reshape(DP, D).T)
        wv = np.ascontiguousarray(
            np.asarray(W_v[hs], np.float32).reshape(DP, D).T)
        wo = np.ascontiguousarray(
            np.asarray(Wo_w, np.float32)[:, DP * c:DP * (c + 1)].T)
        in_maps.append({"xT": xT, "wq": wq, "wk": wk, "wv": wv, "wo": wo,
                        "ones": _ONES})
    return in_maps


def kernel(x, W_q, W_k, W_v, Wo_w, Wo_b):
    nc = _get_program()
    in_maps = make_in_maps(x, W_q, W_k, W_v, Wo_w)
    res = run_bass_kernel_spmd(nc, in_maps, list(range(NCORES)))
    out = res.results[0]["partial"].astype(np.float32)
    for c in range(1, NCORES):
        out += res.results[c]["partial"]
    out += np.asarray(Wo_b, np.float32)
    return out


# revision 34
# speedup vs baseline: 1.0035x; 1.0035x over previous
"""MultiHeadSelfAttention Trainium2 Bass kernel, 8-core SPMD.

Reference:
  q,k,v = einsum('bnd,hkd->bhnk', x, W_{q,k,v});  s = q k^T / sqrt(dk)
  p = softmax(s); out = (p v).transpose -> [B,N,H*DK]; out @ Wo^T + bo

Sharding: head-pair per core (core c owns heads 2c, 2c+1, all batches).
Each core computes a partial output projection over its 128 d-columns of
Wo; host sums the 8 partials and adds the bias.

Numerics: matmuls run as float32r (fp22 operand reads, fp32 PSUM accum).
Softmax row-max comes from an f32r scores pass ([q,m] orientation)
reduced on DVE via tensor_scalar(op1=min) accum; the -max is folded into
the f32r S^T pass as a 65th contraction row, so exp needs no per-q bias.
Denominators come from a ones column appended to V; normalization is
DVE reciprocal -> PE rank-1 broadcast -> ACT stage -> DVE multiply.

Schedule: three interleaved instruction streams per batch window --
attention(b) [st matmuls + exp + pv one step behind], prep(b+1)
[x loads, projections, v transpose, max sweeps], outproj(b-1). All
[128,512] PSUM tiles (S^T halves, sweep chunks, projection slices,
v-transposes, outproj halves) share ONE 6-buffer ring pool, so
execution advances in lockstep with the interleaved emission order --
without the ring, the readiness-greedy scheduler lets attention finish
early and the window ends in a DVE-only sweep-reduce dead zone.
Partition-shift/row DMAs ride the gpsimd (Pool) queue so they never
block the SP DMA stream.
"""
import sys

sys.path.insert(0, "/opt/trn_rl_repo")

from itertools import islice

import numpy as np

import concourse.bass as bass
import concourse.mybir as mybir
import concourse.tile as tile
from concourse import bacc
from concourse.bass_utils import run_bass_kernel_spmd
from concourse.masks import make_identity

B, N, D = 4, 2048, 1024
H, DK = 16, 64
NCORES = 8
HPC = H // NCORES          # heads per core = 2
DP = HPC * DK              # d-slice per core = 128
SCALE = 1.0 / float(np.sqrt(DK))

F32 = mybir.dt.float32
F32R = mybir.dt.float32r
BF16 = mybir.dt.bfloat16

NQT = N // 128             # 16 q tiles per head
NMC = N // 128             # 16 m chunks per head
NHALF = N // 1024          # 2 halves (1024-wide)


def r(ap):
    return ap.bitcast(F32R)


def build_program():
    nc = bacc.Bacc("TRN2", target_bir_lowering=False, debug=False,
                   enable_asserts=False, num_devices=NCORES)

    xT_d = nc.dram_tensor("xT", [B, D, N], F32, kind="ExternalInput")
    wq_d = nc.dram_tensor("wq", [D, DP], F32, kind="ExternalInput")
    wk_d = nc.dram_tensor("wk", [D, DP], F32, kind="ExternalInput")
    wv_d = nc.dram_tensor("wv", [D, DP], F32, kind="ExternalInput")
    wo_d = nc.dram_tensor("wo", [DP, D], F32, kind="ExternalInput")
    ones_d = nc.dram_tensor("ones", [128, N], F32, kind="ExternalInput")
    part_d = nc.dram_tensor("partial", [B, N, D], F32, kind="ExternalOutput")

    with tile.TileContext(nc) as tc:
        build_tile_kernel(nc, tc, xT_d, wq_d, wk_d, wv_d, wo_d, ones_d, part_d)
    nc.compile()
    return nc


def build_tile_kernel(nc, tc, xT_d, wq_d, wk_d, wv_d, wo_d, ones_d, part_d):
    from contextlib import ExitStack
    ctx = ExitStack()
    with ctx:
        # ---- persistent tiles ----
        wpool = ctx.enter_context(tc.tile_pool(name="w", bufs=1))
        # weights stored chunk-major along free dim: [128, 8*128]
        w_sb = {}
        for name, dram in (("wq", wq_d), ("wk", wk_d), ("wv", wv_d)):
            t = wpool.tile([128, D // 128 * DP], F32R, tag=name)
            nc.sync.dma_start(
                out=t[:].rearrange("p (c m) -> p c m", m=DP),
                in_=dram.ap().rearrange("(c p) m -> p c m", p=128).bitcast(F32R),
            )
            w_sb[name] = t
        wo_sb = wpool.tile([DP, D], F32R, tag="wo")
        nc.sync.dma_start(out=wo_sb[:], in_=wo_d.ap()[:].bitcast(F32R))
        id_sb = wpool.tile([128, 128], F32, tag="ident")
        make_identity(nc, id_sb[:])
        ones_sb = wpool.tile([1, 128], F32R, tag="onesrow")
        nc.sync.dma_start(out=ones_sb[:],
                          in_=ones_d.ap()[0:1, 0:128].bitcast(F32R))

        # ---- pools ----
        # PSUM (8 banks): ring 6x1 bank [128,512] shared by every
        # producer/consumer pair; oa 1x2 banks (PV accumulator).
        xt_pool = ctx.enter_context(tc.tile_pool(name="xt", bufs=8))
        ring = ctx.enter_context(tc.tile_pool(name="ring", bufs=6,
                                              space="PSUM"))
        oap = ctx.enter_context(tc.tile_pool(name="oap", bufs=1,
                                             space="PSUM"))
        augp = ctx.enter_context(tc.tile_pool(name="aug", bufs=7))
        vsbp = ctx.enter_context(tc.tile_pool(name="vsb", bufs=1))
        vaugp = ctx.enter_context(tc.tile_pool(name="vaug", bufs=3))
        pp = ctx.enter_context(tc.tile_pool(name="psb", bufs=4))
        attp = ctx.enter_context(tc.tile_pool(name="att", bufs=2))
        tmpp = ctx.enter_context(tc.tile_pool(name="tmp", bufs=2))
        scrp = ctx.enter_context(tc.tile_pool(name="scr", bufs=3))
        nmp = ctx.enter_context(tc.tile_pool(name="nm", bufs=4))
        qkfp = ctx.enter_context(tc.tile_pool(name="qkf", bufs=2))
        outp = ctx.enter_context(tc.tile_pool(name="out", bufs=2))

        state = {}   # per-batch live tiles

        def rtile(name):
            return ring.tile([128, 512], F32, tag="ring", name=name)

        def sweep_qt(h, qt, q_aug, k_aug, negmax, staged):
            """One q-tile of the max sweep: 4 scores chunks + row-max."""
            parts = nmp.tile([128, 4], F32, tag="nmparts")
            for ms in range(4):
                sp = rtile("sq")
                nc.tensor.matmul(
                    sp[:],
                    q_aug[h][0:DK, qt * 128:(qt + 1) * 128],
                    k_aug[h][0:DK, ms * 512:(ms + 1) * 512],
                    start=True, stop=True,
                )
                if ms not in staged:
                    # DVE: fused negate + min-accum straight from PSUM
                    scr = scrp.tile([128, 512], BF16, tag="scr")
                    nc.vector.tensor_scalar(
                        scr[:], sp[:], -1.0, None,
                        mybir.AluOpType.mult, mybir.AluOpType.min,
                        accum_out=parts[:, ms:ms + 1],
                    )
                else:
                    # ACT stages bf16 to SBUF; the DVE reduce then runs in
                    # 4x bf16-SBUF mode (DVE is the scarcer engine)
                    scr = scrp.tile([128, 512], BF16, tag="scr")
                    nc.scalar.activation(
                        scr[:], sp[:],
                        mybir.ActivationFunctionType.Copy,
                        bias=0.0, scale=-1.0)
                    scr2 = scrp.tile([128, 512], BF16, tag="scr",
                                     name="scr2")
                    nc.vector.tensor_scalar(
                        scr2[:], scr[:], 0.0, None,
                        mybir.AluOpType.add, mybir.AluOpType.min,
                        accum_out=parts[:, ms:ms + 1],
                    )
            pdump = nmp.tile([128, 4], F32, tag="nmparts", name="pdump")
            nc.vector.tensor_scalar(
                pdump[:], parts[:, 0:4], 0.0, None,
                mybir.AluOpType.add, mybir.AluOpType.min,
                accum_out=negmax[:, qt:qt + 1],
            )

        def finish_negmax(h, q_aug, negmax):
            """[128,16] -max columns -> aug row 64 via DVE transposes+DMA."""
            nm_t = nmp.tile([32, 128], F32, tag="nmt", name=f"nmt{h}")
            for i in range(4):
                nc.vector.transpose(nm_t[0:32, 32 * i:32 * i + 32],
                                    negmax[32 * i:32 * i + 32, 0:32])
            nc.gpsimd.dma_start(out=q_aug[h][64:65, :],
                                in_=nm_t[0:NQT, :].bitcast(F32R))

        def prep_gen(b, stage_all):
            """Prep for batch b: x loads, projections, v transpose, max
            sweeps. Yields between small units."""
            q_aug = [augp.tile([65, N], F32R, tag="aug", name=f"qaug{h}")
                     for h in range(HPC)]
            k_aug = [augp.tile([65, N], F32R, tag="aug", name=f"kaug{h}")
                     for h in range(HPC)]
            q_f32 = qkfp.tile([128, N], F32, tag="qkf")
            k_f32 = qkfp.tile([128, N], F32, tag="qkf")
            v_sb = vsbp.tile([128, N], F32, tag="vsb")
            st = state[b] = {"q_aug": q_aug, "k_aug": k_aug}

            def evac_qk(aug0, f32stage):
                def evac(psq, half, ns):
                    sl = slice(half * 1024 + ns * 512,
                               half * 1024 + (ns + 1) * 512)
                    # head 0 straight to its aug tile (ACT); head 1
                    # staged (DVE) then partition-shift DMA'd -- one copy
                    # per engine so the ring tile's readers overlap and
                    # neither engine eats both
                    nc.scalar.copy(aug0[0:64, sl], psq[0:64, :])
                    nc.vector.tensor_copy(f32stage[64:128, sl],
                                          psq[64:128, :])
                return evac

            def evac_v(psq, half, ns):
                sl = slice(half * 1024 + ns * 512,
                           half * 1024 + (ns + 1) * 512)
                nc.scalar.copy(v_sb[:, sl], psq[:])

            evacs = [("wq", evac_qk(q_aug[0], q_f32)),
                     ("wk", evac_qk(k_aug[0], k_f32)),
                     ("wv", evac_v)]
            for half in range(NHALF):
                xts = []
                for ch in range(8):
                    xt = xt_pool.tile([128, 1024], F32R, tag="xt")
                    nc.sync.dma_start(
                        out=xt[:],
                        in_=xT_d.ap()[b, ch * 128:(ch + 1) * 128,
                                      half * 1024:(half + 1) * 1024
                                      ].bitcast(F32R),
                    )
                    xts.append(xt)
                    if ch % 2 == 1:
                        yield 0.2
                for tname, evac in evacs:
                    for ns in range(2):
                        psq = rtile("prj")
                        for ch in range(8):
                            nc.tensor.matmul(
                                psq[:],
                                r(w_sb[tname][:, ch * DP:(ch + 1) * DP]),
                                r(xts[ch][:, ns * 512:(ns + 1) * 512]),
                                start=(ch == 0), stop=(ch == 7),
                            )
                        evac(psq, half, ns)
                        yield 0.6

            # head-1 partition shifts + ones rows (gpsimd queue so the SP
            # DMA stream never blocks on aug-tile rotation)
            nc.gpsimd.dma_start(out=q_aug[1][0:64, :],
                                in_=q_f32[64:128, :].bitcast(F32R))
            nc.gpsimd.dma_start(out=k_aug[1][0:64, :],
                                in_=k_f32[64:128, :].bitcast(F32R))
            for h in range(HPC):
                nc.gpsimd.dma_start(out=k_aug[h][64:65, :],
                                    in_=ones_d.ap()[0:1, :].bitcast(F32R))
            yield 0.2

            # v transpose -> v_aug chunks [v^T(64) | ones(64)]
            v_aug = st["v_aug"] = []

            def vaug_head(h):
                va = vaugp.tile([128, NMC * 128], F32R, tag="vaug",
                                name=f"vaug{h}")
                v_aug.append(va)
                hs = slice(h * DK, (h + 1) * DK)
                nc.sync.dma_start(
                    out=va[:].rearrange("p (c w) -> p c w", w=128)[:, :, DK:],
                    in_=ones_d.ap()[:, 0:NMC * DK].rearrange(
                        "p (c w) -> p c w", w=DK).bitcast(F32R))
                for g in range(4):
                    vt_ps = rtile("vt_ps")
                    for j in range(4):
                        mc = g * 4 + j
                        nc.tensor.transpose(
                            vt_ps[:, j * 128:j * 128 + DK],
                            v_sb[hs, mc * 128:(mc + 1) * 128],
                            id_sb[hs, hs])
                    nc.vector.tensor_copy(
                        va[:].rearrange("p (c w) -> p c w",
                                        w=128)[:, g * 4:(g + 1) * 4, 0:DK],
                        vt_ps[:].rearrange("p (c w) -> p c w",
                                           w=128)[:, :, 0:DK])
                    yield 0.6

            yield from vaug_head(0)

            # max sweeps: head 0, head 1; v_aug head 1 last (its pool slot
            # frees only after attention(b-1) head 0 finishes)
            for h in range(HPC):
                negmax = nmp.tile([128, 32], F32, tag="nm",
                                  name=f"negmax{h}")
                nc.vector.memset(negmax[:, NQT:], 0.0)
                for qt in range(NQT):
                    staged = (1, 3) if stage_all else (
                        (3,) if qt % 2 else ())
                    sweep_qt(h, qt, q_aug, k_aug, negmax, staged)
                    yield 2.0
                finish_negmax(h, q_aug, negmax)
                yield 0.5
            yield from vaug_head(1)

        def attn_gen(b):
            """Attention for batch b. PV runs one m-chunk behind exp so the
            PE stream never parks on a fresh exp."""
            q_aug, k_aug = state[b]["q_aug"], state[b]["k_aug"]
            v_aug = state[b]["v_aug"]
            att = attp.tile([128, N], F32R, tag="att")
            state[b]["att"] = att

            def emit_pv(h, oa, ps, mc):
                for qs in range(2):
                    nc.tensor.matmul(
                        oa[:, qs * 512:(qs + 1) * 512],
                        r(v_aug[h][:, mc * 128:(mc + 1) * 128]),
                        r(ps[qs][:]),
                        start=(mc == 0), stop=(mc == NMC - 1),
                    )

            for h in range(HPC):
                for qh in range(NHALF):
                    qsl = slice(qh * 1024, (qh + 1) * 1024)
                    oa = oap.tile([128, 1024], F32, tag="oa", name="oa")
                    prev = None  # (p_pair, mc) pending PV
                    for mc in range(NMC):
                        ps = []
                        for qs in range(2):
                            stt = rtile("st")
                            nc.tensor.matmul(
                                stt[:],
                                r(k_aug[h][:, mc * 128:(mc + 1) * 128]),
                                r(q_aug[h][:, qh * 1024 + qs * 512:
                                           qh * 1024 + (qs + 1) * 512]),
                                start=True, stop=True,
                            )
                            p_sb = pp.tile([128, 512], F32R, tag="psb")
                            nc.scalar.activation(
                                p_sb[:], stt[:],
                                mybir.ActivationFunctionType.Exp,
                                bias=0.0, scale=SCALE)
                            ps.append(p_sb)
                        if prev is not None:
                            emit_pv(h, oa, *prev)
                        prev = (ps, mc)
                        yield
                    emit_pv(h, oa, *prev)
                    # normalize: att rows = oa[0:64] * (1/denom); denom on
                    # oa[64:65] via the ones columns of v_aug. DVE has no
                    # divide ISA op and engines read at most one PSUM
                    # operand, so: DVE reciprocal -> PE rank-1 broadcast
                    # (ring) -> ACT copy to SBUF -> DVE mult.
                    recip = tmpp.tile([1, 1024], F32R, tag="recip")
                    with nc.allow_low_precision(reason="f32r is 4-byte"):
                        nc.vector.reciprocal(recip[:], oa[64:65, :])
                    rbc = tmpp.tile([64, 1024], F32R, tag="atmp",
                                    name="rbc")
                    for qs in range(2):
                        rb = rtile("rb")
                        nc.tensor.matmul(
                            rb[0:64, :],
                            r(ones_sb[0:1, 0:64]),
                            r(recip[0:1, qs * 512:(qs + 1) * 512]),
                            start=True, stop=True,
                        )
                        nc.scalar.copy(rbc[:, qs * 512:(qs + 1) * 512],
                                       rb[0:64, :])
                    if h == 0:
                        nc.vector.tensor_tensor(
                            att[0:64, qsl], oa[0:64, :], rbc[:],
                            mybir.AluOpType.mult)
                    else:
                        atmp = tmpp.tile([64, 1024], F32R, tag="atmp",
                                         name="atmp")
                        nc.vector.tensor_tensor(
                            atmp[:], oa[0:64, :], rbc[:],
                            mybir.AluOpType.mult)
                        # partition shift 0-63 -> 64-127
                        nc.gpsimd.dma_start(out=att[64:128, qsl],
                                            in_=atmp[:])
                    yield

        def outproj_gen(b, dve_only=False):
            att = state[b]["att"]
            for nt in range(N // 128):
                ostg = outp.tile([128, 1024], F32, tag="ostg")
                for es in range(2):
                    op = rtile("op")
                    nc.tensor.matmul(
                        op[:],
                        r(att[:, nt * 128:(nt + 1) * 128]),
                        r(wo_sb[:, es * 512:(es + 1) * 512]),
                        start=True, stop=True,
                    )
                    if es == 0 and not dve_only:
                        # ACT takes one half except in the last window,
                        # where exp already saturates ACT
                        nc.scalar.copy(ostg[:, 0:512], op[:])
                    else:
                        nc.vector.tensor_copy(
                            ostg[:, es * 512:(es + 1) * 512], op[:])
                nc.sync.dma_start(
                    out=part_d.ap()[b, nt * 128:(nt + 1) * 128, :],
                    in_=ostg[:],
                )
                yield

        # ---- drive the interleaved windows ----
        A_STEPS = HPC * NHALF * (NMC + 1)          # 68
        P_WEIGHT = 8 * 0.2 + 12 * 0.6 + 0.2 + 8 * 0.6 + 32 * 2.0 + 2 * 0.5
        # prologue: prep(0) alone through sweep head 0 (step 42), then its
        # remainder interleaves with early attention(0)
        p0 = prep_gen(0, stage_all=True)
        for _ in islice(p0, 42):
            pass
        a_cur = attn_gen(0)
        a_consumed = 0
        for _ in p0:
            next(a_cur, None)
            a_consumed += 1

        _sent = object()
        for b in range(B):
            a = a_cur
            a_rem = max(1, A_STEPS - (a_consumed if b == 0 else 0))
            p = prep_gen(b + 1, stage_all=False) if b + 1 < B else None
            o = outproj_gen(b - 1, dve_only=(b == B - 1)) if b >= 1 else None
            o_self = None
            i = 0

            def pull_a():
                # one attention step plus its share of the outproj streams
                nonlocal i, o_self
                if next(a, _sent) is _sent:
                    return False
                if o is not None and i % 4 == 0:
                    next(o, None)
                if b == B - 1 and i >= a_rem - 16:
                    # last window: batch-3 outproj columns for the first
                    # q-half become valid once head-1/qh0 finishes
                    if o_self is None:
                        o_self = outproj_gen(b, dve_only=True)
                    if i % 2 == 0:
                        next(o_self, None)
                i += 1
                return True

            if p is not None:
                # pace attention by prep weight: a sweep step carries 4x
                # the ring tiles of a projection step
                acc = 0.0
                alive = True
                for w in p:
                    acc += w * (a_rem / P_WEIGHT)
                    while acc >= 1.0:
                        acc -= 1.0
                        if alive:
                            alive = pull_a()
            while pull_a():
                pass
            if o is not None:
                for _ in o:
                    pass
            a_cur = attn_gen(b + 1) if b + 1 < B else None
        for _ in (o_self or outproj_gen(B - 1, dve_only=True)):
            pass


_PROGRAM = None


def _get_program():
    global _PROGRAM
    if _PROGRAM is None:
        _PROGRAM = build_program()
    return _PROGRAM


_ONES = np.ones((128, N), np.float32)


def make_in_maps(x, W_q, W_k, W_v, Wo_w):
    xT = np.ascontiguousarray(np.transpose(
        np.asarray(x, np.float32), (0, 2, 1)))
    in_maps = []
    for c in range(NCORES):
        hs = slice(HPC * c, HPC * (c + 1))
        wq = np.ascontiguousarray(
            np.asarray(W_q[hs], np.float32).reshape(DP, D).T)
        wk = np.ascontiguousarray(
            np.asarray(W_k[hs], np.float32).reshape(DP, D).T)
        wv = np.ascontiguousarray(
            np.asarray(W_v[hs], np.float32).reshape(DP, D).T)
        wo = np.ascontiguousarray(
            np.asarray(Wo_w, np.float32)[:, DP * c:DP * (c + 1)].T)
        in_maps.append({"xT": xT, "wq": wq, "wk": wk, "wv": wv, "wo": wo,
                        "ones": _ONES})
    return in_maps


def kernel(x, W_q, W_k, W_v, Wo_w, Wo_b):
    nc = _get_program()
    in_maps = make_in_maps(x, W_q, W_k, W_v, Wo_w)
    res = run_bass_kernel_spmd(nc, in_maps, list(range(NCORES)))
    out = res.results[0]["partial"].astype(np.float32)
    for c in range(1, NCORES):
        out += res.results[c]["partial"]
    out += np.asarray(Wo_b, np.float32)
    return out


# revision 35
# speedup vs baseline: 1.0042x; 1.0007x over previous
"""MultiHeadSelfAttention Trainium2 Bass kernel, 8-core SPMD.

Reference:
  q,k,v = einsum('bnd,hkd->bhnk', x, W_{q,k,v});  s = q k^T / sqrt(dk)
  p = softmax(s); out = (p v).transpose -> [B,N,H*DK]; out @ Wo^T + bo

Sharding: head-pair per core (core c owns heads 2c, 2c+1, all batches).
Each core computes a partial output projection over its 128 d-columns of
Wo; host sums the 8 partials and adds the bias.

Numerics: matmuls run as float32r (fp22 operand reads, fp32 PSUM accum).
Softmax row-max comes from an f32r scores pass ([q,m] orientation)
reduced on DVE via tensor_scalar(op1=min) accum; the -max is folded into
the f32r S^T pass as a 65th contraction row, so exp needs no per-q bias.
Denominators come from a ones column appended to V; normalization is
DVE reciprocal -> PE rank-1 broadcast -> ACT stage -> DVE multiply.

Schedule: three interleaved instruction streams per batch window --
attention(b) [st matmuls + exp + pv one step behind], prep(b+1)
[x loads, projections, v transpose, max sweeps], outproj(b-1). All
[128,512] PSUM tiles (S^T halves, sweep chunks, projection slices,
v-transposes, outproj halves) share ONE 6-buffer ring pool, so
execution advances in lockstep with the interleaved emission order --
without the ring, the readiness-greedy scheduler lets attention finish
early and the window ends in a DVE-only sweep-reduce dead zone.
Partition-shift/row DMAs ride the gpsimd (Pool) queue so they never
block the SP DMA stream.
"""
import sys

sys.path.insert(0, "/opt/trn_rl_repo")

from itertools import islice

import numpy as np

import concourse.bass as bass
import concourse.mybir as mybir
import concourse.tile as tile
from concourse import bacc
from concourse.bass_utils import run_bass_kernel_spmd
from concourse.masks import make_identity

B, N, D = 4, 2048, 1024
H, DK = 16, 64
NCORES = 8
HPC = H // NCORES          # heads per core = 2
DP = HPC * DK              # d-slice per core = 128
SCALE = 1.0 / float(np.sqrt(DK))

F32 = mybir.dt.float32
F32R = mybir.dt.float32r
BF16 = mybir.dt.bfloat16

NQT = N // 128             # 16 q tiles per head
NMC = N // 128             # 16 m chunks per head
NHALF = N // 1024          # 2 halves (1024-wide)


def r(ap):
    return ap.bitcast(F32R)


def build_program():
    nc = bacc.Bacc("TRN2", target_bir_lowering=False, debug=False,
                   enable_asserts=False, num_devices=NCORES)

    xT_d = nc.dram_tensor("xT", [B, D, N], F32, kind="ExternalInput")
    wq_d = nc.dram_tensor("wq", [D, DP], F32, kind="ExternalInput")
    wk_d = nc.dram_tensor("wk", [D, DP], F32, kind="ExternalInput")
    wv_d = nc.dram_tensor("wv", [D, DP], F32, kind="ExternalInput")
    wo_d = nc.dram_tensor("wo", [DP, D], F32, kind="ExternalInput")
    ones_d = nc.dram_tensor("ones", [128, N], F32, kind="ExternalInput")
    part_d = nc.dram_tensor("partial", [B, N, D], F32, kind="ExternalOutput")

    with tile.TileContext(nc) as tc:
        build_tile_kernel(nc, tc, xT_d, wq_d, wk_d, wv_d, wo_d, ones_d, part_d)
    nc.compile()
    return nc


def build_tile_kernel(nc, tc, xT_d, wq_d, wk_d, wv_d, wo_d, ones_d, part_d):
    from contextlib import ExitStack
    ctx = ExitStack()
    with ctx:
        # ---- persistent tiles ----
        wpool = ctx.enter_context(tc.tile_pool(name="w", bufs=1))
        # weights stored chunk-major along free dim: [128, 8*128]
        w_sb = {}
        for name, dram in (("wq", wq_d), ("wk", wk_d), ("wv", wv_d)):
            t = wpool.tile([128, D // 128 * DP], F32R, tag=name)
            nc.sync.dma_start(
                out=t[:].rearrange("p (c m) -> p c m", m=DP),
                in_=dram.ap().rearrange("(c p) m -> p c m", p=128).bitcast(F32R),
            )
            w_sb[name] = t
        wo_sb = wpool.tile([DP, D], F32R, tag="wo")
        nc.sync.dma_start(out=wo_sb[:], in_=wo_d.ap()[:].bitcast(F32R))
        id_sb = wpool.tile([128, 128], F32, tag="ident")
        make_identity(nc, id_sb[:])
        ones_sb = wpool.tile([1, 128], F32R, tag="onesrow")
        nc.sync.dma_start(out=ones_sb[:],
                          in_=ones_d.ap()[0:1, 0:128].bitcast(F32R))

        # ---- pools ----
        # PSUM (8 banks): ring 6x1 bank [128,512] shared by every
        # producer/consumer pair; oa 1x2 banks (PV accumulator).
        xt_pool = ctx.enter_context(tc.tile_pool(name="xt", bufs=8))
        ring = ctx.enter_context(tc.tile_pool(name="ring", bufs=6,
                                              space="PSUM"))
        oap = ctx.enter_context(tc.tile_pool(name="oap", bufs=1,
                                             space="PSUM"))
        augp = ctx.enter_context(tc.tile_pool(name="aug", bufs=7))
        vsbp = ctx.enter_context(tc.tile_pool(name="vsb", bufs=1))
        vaugp = ctx.enter_context(tc.tile_pool(name="vaug", bufs=3))
        pp = ctx.enter_context(tc.tile_pool(name="psb", bufs=4))
        attp = ctx.enter_context(tc.tile_pool(name="att", bufs=2))
        tmpp = ctx.enter_context(tc.tile_pool(name="tmp", bufs=2))
        scrp = ctx.enter_context(tc.tile_pool(name="scr", bufs=3))
        nmp = ctx.enter_context(tc.tile_pool(name="nm", bufs=4))
        qkfp = ctx.enter_context(tc.tile_pool(name="qkf", bufs=2))
        outp = ctx.enter_context(tc.tile_pool(name="out", bufs=2))

        state = {}   # per-batch live tiles

        def rtile(name):
            return ring.tile([128, 512], F32, tag="ring", name=name)

        def sweep_qt(h, qt, q_aug, k_aug, negmax, staged):
            """One q-tile of the max sweep: 4 scores chunks + row-max."""
            parts = nmp.tile([128, 4], F32, tag="nmparts")
            for ms in range(4):
                sp = rtile("sq")
                nc.tensor.matmul(
                    sp[:],
                    q_aug[h][0:DK, qt * 128:(qt + 1) * 128],
                    k_aug[h][0:DK, ms * 512:(ms + 1) * 512],
                    start=True, stop=True,
                )
                if ms not in staged:
                    # DVE: fused negate + min-accum straight from PSUM
                    scr = scrp.tile([128, 512], BF16, tag="scr")
                    nc.vector.tensor_scalar(
                        scr[:], sp[:], -1.0, None,
                        mybir.AluOpType.mult, mybir.AluOpType.min,
                        accum_out=parts[:, ms:ms + 1],
                    )
                else:
                    # ACT stages bf16 to SBUF; the DVE reduce then runs in
                    # 4x bf16-SBUF mode (DVE is the scarcer engine)
                    scr = scrp.tile([128, 512], BF16, tag="scr")
                    nc.scalar.activation(
                        scr[:], sp[:],
                        mybir.ActivationFunctionType.Copy,
                        bias=0.0, scale=-1.0)
                    scr2 = scrp.tile([128, 512], BF16, tag="scr",
                                     name="scr2")
                    nc.vector.tensor_scalar(
                        scr2[:], scr[:], 0.0, None,
                        mybir.AluOpType.add, mybir.AluOpType.min,
                        accum_out=parts[:, ms:ms + 1],
                    )
            pdump = nmp.tile([128, 4], F32, tag="nmparts", name="pdump")
            nc.vector.tensor_scalar(
                pdump[:], parts[:, 0:4], 0.0, None,
                mybir.AluOpType.add, mybir.AluOpType.min,
                accum_out=negmax[:, qt:qt + 1],
            )

        def finish_negmax(h, q_aug, negmax):
            """[128,16] -max columns -> aug row 64 via DVE transposes+DMA."""
            nm_t = nmp.tile([32, 128], F32, tag="nmt", name=f"nmt{h}")
            for i in range(4):
                nc.vector.transpose(nm_t[0:32, 32 * i:32 * i + 32],
                                    negmax[32 * i:32 * i + 32, 0:32])
            nc.gpsimd.dma_start(out=q_aug[h][64:65, :],
                                in_=nm_t[0:NQT, :].bitcast(F32R))

        def prep_gen(b, stage_all):
            """Prep for batch b: x loads, projections, v transpose, max
            sweeps. Yields between small units."""
            q_aug = [augp.tile([65, N], F32R, tag="aug", name=f"qaug{h}")
                     for h in range(HPC)]
            k_aug = [augp.tile([65, N], F32R, tag="aug", name=f"kaug{h}")
                     for h in range(HPC)]
            q_f32 = qkfp.tile([128, N], F32, tag="qkf")
            k_f32 = qkfp.tile([128, N], F32, tag="qkf")
            v_sb = vsbp.tile([128, N], F32, tag="vsb")
            st = state[b] = {"q_aug": q_aug, "k_aug": k_aug}

            def evac_qk(aug0, f32stage):
                def evac(psq, half, ns):
                    sl = slice(half * 1024 + ns * 512,
                               half * 1024 + (ns + 1) * 512)
                    # head 0 straight to its aug tile (ACT); head 1
                    # staged (DVE) then partition-shift DMA'd -- one copy
                    # per engine so the ring tile's readers overlap and
                    # neither engine eats both
                    nc.scalar.copy(aug0[0:64, sl], psq[0:64, :])
                    nc.vector.tensor_copy(f32stage[64:128, sl],
                                          psq[64:128, :])
                return evac

            def evac_v(psq, half, ns):
                sl = slice(half * 1024 + ns * 512,
                           half * 1024 + (ns + 1) * 512)
                nc.scalar.copy(v_sb[:, sl], psq[:])

            evacs = [("wq", evac_qk(q_aug[0], q_f32)),
                     ("wk", evac_qk(k_aug[0], k_f32)),
                     ("wv", evac_v)]
            for half in range(NHALF):
                xts = []
                for ch in range(8):
                    xt = xt_pool.tile([128, 1024], F32R, tag="xt")
                    nc.sync.dma_start(
                        out=xt[:],
                        in_=xT_d.ap()[b, ch * 128:(ch + 1) * 128,
                                      half * 1024:(half + 1) * 1024
                                      ].bitcast(F32R),
                    )
                    xts.append(xt)
                    if ch % 2 == 1:
                        yield 0.2
                for tname, evac in evacs:
                    for ns in range(2):
                        psq = rtile("prj")
                        for ch in range(8):
                            nc.tensor.matmul(
                                psq[:],
                                r(w_sb[tname][:, ch * DP:(ch + 1) * DP]),
                                r(xts[ch][:, ns * 512:(ns + 1) * 512]),
                                start=(ch == 0), stop=(ch == 7),
                            )
                        evac(psq, half, ns)
                        yield 0.6

            # head-1 partition shifts + ones rows (gpsimd queue so the SP
            # DMA stream never blocks on aug-tile rotation)
            nc.gpsimd.dma_start(out=q_aug[1][0:64, :],
                                in_=q_f32[64:128, :].bitcast(F32R))
            nc.gpsimd.dma_start(out=k_aug[1][0:64, :],
                                in_=k_f32[64:128, :].bitcast(F32R))
            for h in range(HPC):
                nc.gpsimd.dma_start(out=k_aug[h][64:65, :],
                                    in_=ones_d.ap()[0:1, :].bitcast(F32R))
            yield 0.2

            # v transpose -> v_aug chunks [v^T(64) | ones(64)]
            v_aug = st["v_aug"] = []

            def vaug_head(h):
                va = vaugp.tile([128, NMC * 128], F32R, tag="vaug",
                                name=f"vaug{h}")
                v_aug.append(va)
                hs = slice(h * DK, (h + 1) * DK)
                nc.sync.dma_start(
                    out=va[:].rearrange("p (c w) -> p c w", w=128)[:, :, DK:],
                    in_=ones_d.ap()[:, 0:NMC * DK].rearrange(
                        "p (c w) -> p c w", w=DK).bitcast(F32R))
                for g in range(4):
                    vt_ps = rtile("vt_ps")
                    for j in range(4):
                        mc = g * 4 + j
                        nc.tensor.transpose(
                            vt_ps[:, j * 128:j * 128 + DK],
                            v_sb[hs, mc * 128:(mc + 1) * 128],
                            id_sb[hs, hs])
                    nc.vector.tensor_copy(
                        va[:].rearrange("p (c w) -> p c w",
                                        w=128)[:, g * 4:(g + 1) * 4, 0:DK],
                        vt_ps[:].rearrange("p (c w) -> p c w",
                                           w=128)[:, :, 0:DK])
                    yield 0.6

            yield from vaug_head(0)

            # max sweeps: head 0, head 1; v_aug head 1 last (its pool slot
            # frees only after attention(b-1) head 0 finishes)
            for h in range(HPC):
                negmax = nmp.tile([128, 32], F32, tag="nm",
                                  name=f"negmax{h}")
                nc.vector.memset(negmax[:, NQT:], 0.0)
                for qt in range(NQT):
                    staged = (1, 3) if stage_all else (
                        (3,) if qt % 2 else ())
                    sweep_qt(h, qt, q_aug, k_aug, negmax, staged)
                    yield 2.0
                finish_negmax(h, q_aug, negmax)
                yield 0.5
            yield from vaug_head(1)

        def attn_gen(b):
            """Attention for batch b. PV runs one m-chunk behind exp so the
            PE stream never parks on a fresh exp."""
            q_aug, k_aug = state[b]["q_aug"], state[b]["k_aug"]
            v_aug = state[b]["v_aug"]
            att = attp.tile([128, N], F32R, tag="att")
            state[b]["att"] = att

            def emit_pv(h, oa, ps, mc):
                for qs in range(2):
                    nc.tensor.matmul(
                        oa[:, qs * 512:(qs + 1) * 512],
                        r(v_aug[h][:, mc * 128:(mc + 1) * 128]),
                        r(ps[qs][:]),
                        start=(mc == 0), stop=(mc == NMC - 1),
                    )

            for h in range(HPC):
                for qh in range(NHALF):
                    qsl = slice(qh * 1024, (qh + 1) * 1024)
                    oa = oap.tile([128, 1024], F32, tag="oa", name="oa")
                    prev = None  # (p_pair, mc) pending PV
                    for mc in range(NMC):
                        ps = []
                        for qs in range(2):
                            stt = rtile("st")
                            nc.tensor.matmul(
                                stt[:],
                                r(k_aug[h][:, mc * 128:(mc + 1) * 128]),
                                r(q_aug[h][:, qh * 1024 + qs * 512:
                                           qh * 1024 + (qs + 1) * 512]),
                                start=True, stop=True,
                            )
                            p_sb = pp.tile([128, 512], F32R, tag="psb")
                            nc.scalar.activation(
                                p_sb[:], stt[:],
                                mybir.ActivationFunctionType.Exp,
                                bias=0.0, scale=SCALE)
                            ps.append(p_sb)
                        if prev is not None:
                            emit_pv(h, oa, *prev)
                        prev = (ps, mc)
                        yield
                    emit_pv(h, oa, *prev)
                    # normalize: att rows = oa[0:64] * (1/denom); denom on
                    # oa[64:65] via the ones columns of v_aug. DVE has no
                    # divide ISA op and engines read at most one PSUM
                    # operand, so: DVE reciprocal -> PE rank-1 broadcast
                    # (ring) -> ACT copy to SBUF -> DVE mult.
                    recip = tmpp.tile([1, 1024], F32R, tag="recip")
                    with nc.allow_low_precision(reason="f32r is 4-byte"):
                        nc.vector.reciprocal(recip[:], oa[64:65, :])
                    rbc = tmpp.tile([64, 1024], F32R, tag="atmp",
                                    name="rbc")
                    for qs in range(2):
                        rb = rtile("rb")
                        nc.tensor.matmul(
                            rb[0:64, :],
                            r(ones_sb[0:1, 0:64]),
                            r(recip[0:1, qs * 512:(qs + 1) * 512]),
                            start=True, stop=True,
                        )
                        nc.scalar.copy(rbc[:, qs * 512:(qs + 1) * 512],
                                       rb[0:64, :])
                    if h == 0:
                        nc.vector.tensor_tensor(
                            att[0:64, qsl], oa[0:64, :], rbc[:],
                            mybir.AluOpType.mult)
                    else:
                        atmp = tmpp.tile([64, 1024], F32R, tag="atmp",
                                         name="atmp")
                        nc.vector.tensor_tensor(
                            atmp[:], oa[0:64, :], rbc[:],
                            mybir.AluOpType.mult)
                        # partition shift 0-63 -> 64-127
                        nc.gpsimd.dma_start(out=att[64:128, qsl],
                                            in_=atmp[:])
                    yield

        def outproj_gen(b, dve_only=False):
            att = state[b]["att"]
            for nt in range(N // 128):
                ostg = outp.tile([128, 1024], F32, tag="ostg")
                for es in range(2):
                    op = rtile("op")
                    nc.tensor.matmul(
                        op[:],
                        r(att[:, nt * 128:(nt + 1) * 128]),
                        r(wo_sb[:, es * 512:(es + 1) * 512]),
                        start=True, stop=True,
                    )
                    if es == 0 and not dve_only:
                        # ACT takes one half except in the last window,
                        # where exp already saturates ACT
                        nc.scalar.copy(ostg[:, 0:512], op[:])
                    else:
                        nc.vector.tensor_copy(
                            ostg[:, es * 512:(es + 1) * 512], op[:])
                nc.sync.dma_start(
                    out=part_d.ap()[b, nt * 128:(nt + 1) * 128, :],
                    in_=ostg[:],
                )
                yield

        # ---- drive the interleaved windows ----
        A_STEPS = HPC * NHALF * (NMC + 1)          # 68
        P_WEIGHT = 8 * 0.2 + 12 * 0.6 + 0.2 + 8 * 0.6 + 32 * 2.0 + 2 * 0.5
        # prologue: prep(0) alone through sweep head 0 (step 42), then its
        # remainder interleaves with early attention(0)
        p0 = prep_gen(0, stage_all=True)
        for _ in islice(p0, 42):
            pass
        a_cur = attn_gen(0)
        a_consumed = 0
        for _ in p0:
            next(a_cur, None)
            a_consumed += 1

        _sent = object()
        for b in range(B):
            a = a_cur
            a_rem = max(1, A_STEPS - (a_consumed if b == 0 else 0))
            p = prep_gen(b + 1, stage_all=False) if b + 1 < B else None
            o = outproj_gen(b - 1, dve_only=(b == B - 1)) if b >= 1 else None
            o_self = None
            i = 0

            def pull_a():
                # one attention step plus its share of the outproj streams
                nonlocal i, o_self
                if next(a, _sent) is _sent:
                    return False
                if o is not None and i >= 12 and i % 3 == 0:
                    # defer outproj past the projection stretch: its ACT
                    # evac half would pile onto the exp+evac-saturated
                    # window opening
                    next(o, None)
                if b == B - 1 and i >= a_rem - 16:
                    # last window: batch-3 outproj columns for the first
                    # q-half become valid once head-1/qh0 finishes
                    if o_self is None:
                        o_self = outproj_gen(b, dve_only=True)
                    if i % 2 == 0:
                        next(o_self, None)
                i += 1
                return True

            if p is not None:
                # pace attention by prep weight: a sweep step carries 4x
                # the ring tiles of a projection step
                acc = 0.0
                alive = True
                for w in p:
                    acc += w * (a_rem / P_WEIGHT)
                    while acc >= 1.0:
                        acc -= 1.0
                        if alive:
                            alive = pull_a()
            while pull_a():
                pass
            if o is not None:
                for _ in o:
                    pass
            a_cur = attn_gen(b + 1) if b + 1 < B else None
        for _ in (o_self or outproj_gen(B - 1, dve_only=True)):
            pass


_PROGRAM = None


def _get_program():
    global _PROGRAM
    if _PROGRAM is None:
        _PROGRAM = build_program()
    return _PROGRAM


_ONES = np.ones((128, N), np.float32)


def make_in_maps(x, W_q, W_k, W_v, Wo_w):
    xT = np.ascontiguousarray(np.transpose(
        np.asarray(x, np.float32), (0, 2, 1)))
    in_maps = []
    for c in range(NCORES):
        hs = slice(HPC * c, HPC * (c + 1))
        wq = np.ascontiguousarray(
            np.asarray(W_q[hs], np.float32).reshape(DP, D).T)
        wk = np.ascontiguousarray(
            np.asarray(W_k[hs], np.float32).reshape(DP, D).T)
        wv = np.ascontiguousarray(
            np.asarray(W_v[hs], np.float32).reshape(DP, D).T)
        wo = np.ascontiguousarray(
            np.asarray(Wo_w, np.float32)[:, DP * c:DP * (c + 1)].T)
        in_maps.append({"xT": xT, "wq": wq, "wk": wk, "wv": wv, "wo": wo,
                        "ones": _ONES})
    return in_maps


def kernel(x, W_q, W_k, W_v, Wo_w, Wo_b):
    nc = _get_program()
    in_maps = make_in_maps(x, W_q, W_k, W_v, Wo_w)
    res = run_bass_kernel_spmd(nc, in_maps, list(range(NCORES)))
    out = res.results[0]["partial"].astype(np.float32)
    for c in range(1, NCORES):
        out += res.results[c]["partial"]
    out += np.asarray(Wo_b, np.float32)
    return out
